# revision 21
# baseline (speedup 1.0000x reference)
"""Multi-head GAT layer for Trainium2 — 8 heads sharded across 8 NeuronCores.

Per head h (N=4096 nodes, F=64 features):
    ltg   = graph @ W[h]                          [N, F]
    s     = ltg @ a_src,  d = ltg @ a_dst         [N]
    E     = leaky_relu(s[:, None] + d[None, :], 0.2)
    Alpha = softmax(E, axis=-1)
    out   = Alpha @ ltg

Decomposition: with z = s_i + d_j and M_ij = [z >= 0],

    exp(leaky_relu(z)) = M_ij e^{s_i} e^{d_j} + (1-M_ij) e^{.2 s_i} e^{.2 d_j}

so softmax reduces to one mask pass plus masked matmuls on the PE:

    eps1 = M @ [v.*ltg | v]      (v  = e^{d})
    eps2 = M @ [v2.*ltg | v2]    (v2 = e^{.2 d})
    q    = rho * eps1 - eps2     (rho = e^{.8 s}, per row i)
    out  = (q[:, :F] + T2) / (q[:, F] + t2)

[T2 | t2] are full column sums of [v2.*ltg | v2]; the complement path is
total-minus-masked with identical bf16 summands, so it is exact.

Masks are made on two engines, two supers (1024 i-columns) at a time:
  - DVE tensor_scalar(is_ge) -> {0,1} masks, blocks B0.
  - ScalarE activation(Sign, bias=d_j) -> {-1,0,+1} masks tau, blocks B±.
    Sign is present in every activation table set, so no table reloads.
    For B± blocks R is stored halved; M @ R = tau @ (R/2) + ones @ (R/2),
    handled by one extra ones-stationary matmul per accumulation group
    against RS = sum of the halved R blocks.  At z == 0 sign gives 0,
    i.e. the average of both paths — exact, since they coincide there.

Heads are fully independent: core h computes head h; no collectives.
"""

import os
from contextlib import ExitStack

import numpy as np

N, F_IN, F, H = 4096, 64, 64, 8
P = 128
NB = N // P           # 32 node blocks (j)
ISUP = 4              # i-blocks per super (one PSUM acc generation)
NSUP = NB // ISUP     # 8 supers
NPAIR = NSUP // 2     # mask tiles span a pair of supers
RC = 130              # R columns per j-block: [v.*ltg | v | v2.*ltg | v2]
# blocks with (b % 8) in SC_PAT get ScalarE sign masks; rest DVE is_ge
SC_PAT = tuple(int(c) for c in os.environ.get("GAT_SC_PAT", "036"))
_CACHE = {}


def _build():
    import concourse.bass as bass  # noqa: F401
    import concourse.mybir as mybir
    import concourse.tile as tile
    from concourse import bacc

    dt = mybir.dt
    f32 = dt.float32
    bf16 = dt.bfloat16
    Alu = mybir.AluOpType
    Act = mybir.ActivationFunctionType

    nc = bacc.Bacc("TRN2", debug=False, num_devices=H)
    graph_d = nc.dram_tensor("graph", [N, F_IN], f32, kind="ExternalInput").ap()
    w_d = nc.dram_tensor("w", [F_IN, F], f32, kind="ExternalInput").ap()
    a_d = nc.dram_tensor("a", [2, F], f32, kind="ExternalInput").ap()
    out_d = nc.dram_tensor("out", [N, F], f32, kind="ExternalOutput").ap()

    ident_d = nc.inline_tensor(np.eye(P, dtype=np.float32), name="ident")
    is_sc = [1 if (b % 8) in SC_PAT else 0 for b in range(NB)]
    hrow_d = nc.inline_tensor(
        np.repeat(np.where(np.array(is_sc), 0.5, 1.0)[None, :],
                  P, axis=0).astype(np.float32), name="hrow")

    with tile.TileContext(nc) as tc, ExitStack() as ctx:
        persist = ctx.enter_context(tc.tile_pool(name="persist", bufs=1))
        sps = ctx.enter_context(tc.tile_pool(name="sps", bufs=2, space="PSUM"))
        accp = ctx.enter_context(tc.tile_pool(name="acc", bufs=2, space="PSUM"))
        gp = ctx.enter_context(tc.tile_pool(name="gp", bufs=3))
        mp = ctx.enter_context(tc.tile_pool(name="mask", bufs=2))
        ep = ctx.enter_context(tc.tile_pool(name="ep", bufs=2))

        identity = persist.tile([P, P], f32)
        nc.sync.dma_start(identity[:], ident_d.ap())
        hmul = persist.tile([P, NB], f32)            # 0.5 on B±, 1.0 on B0
        nc.sync.dma_start(hmul[:], hrow_d.ap())
        ones_col_bf = persist.tile([P, 1], bf16)
        nc.vector.memset(ones_col_bf[:], 1.0)
        ones_row = persist.tile([1, P], f32)
        nc.vector.memset(ones_row[:], 1.0)
        ones64 = persist.tile([F_IN, P], f32)
        nc.vector.memset(ones64[:], 1.0)
        ones_sq_bf = persist.tile([P, P], bf16)      # stationary for RS matmul
        nc.vector.memset(ones_sq_bf[:], 1.0)

        # fused [W | w_s | w_d] rhs for the per-block projection matmul
        wssd = persist.tile([F_IN, F + 2], f32)
        nc.sync.dma_start(wssd[:, 0:F], w_d[:])
        a2_sb = persist.tile([F, 2], f32)
        nc.sync.dma_start(a2_sb[:], a_d.rearrange("t k -> k t"))

        gT = persist.tile([F_IN, N], f32)            # graph^T
        ltgsd = persist.tile([P, 66 * NB], f32)      # per b: ltg (64) | s | d
        negd = persist.tile([P, NB], f32)            # -d columns
        vuse = persist.tile([P, NB], f32)            # e^{d}   (halved on B±)
        v2use = persist.tile([P, NB], f32)           # e^{.2d} (halved on B±)
        rho = persist.tile([P, NB], f32)             # e^{.8 s}
        s_rep = persist.tile([P, N], bf16)           # s broadcast down partitions
        r_all = persist.tile([P, RC * NB], bf16)     # [R1|v|R2|v2] per b
        RS = persist.tile([P, RC], bf16)             # sum of B± (halved) R
        t2rep = persist.tile([P, F + 1], f32)        # [T2 | t2] bcast down parts

        ltgsd_v = ltgsd.rearrange("p (b c) -> p b c", c=66)
        r_v = r_all.rearrange("p (b c) -> p b c", c=RC)

        # W^T, then [w_s | w_d] = W^T.T @ a2
        wT_ps = sps.tile([F, F_IN], f32, tag="tp")
        nc.tensor.transpose(wT_ps[:], wssd[:, 0:F], identity[0:F_IN, 0:F_IN])
        wT_sb = gp.tile([F, F_IN], f32, tag="wT")
        nc.vector.tensor_copy(wT_sb[:], wT_ps[:])
        wsd_ps = sps.tile([F_IN, 2], f32, tag="pj")
        nc.tensor.matmul(wsd_ps[:], wT_sb[:], a2_sb[:])
        nc.vector.tensor_copy(wssd[:, F:F + 2], wsd_ps[:])
        # W1[k, p] = w_s[k]  (pre-broadcast stationary for s_rep chunks)
        W1 = persist.tile([F_IN, P], f32)
        nc.vector.tensor_scalar(W1[:], ones64[:], wssd[:, F:F + 1], None,
                                op0=Alu.mult)

        mask_tiles = {}

        def emit_mask(pair, b):
            i0 = pair * 2 * ISUP * P
            w = 2 * ISUP * P
            mt = mp.tile([P, w], bf16, tag=f"m{b}", name=f"mask{b}")
            if is_sc[b]:
                nc.scalar.activation(mt[:], s_rep[:, i0:i0 + w], Act.Sign,
                                     bias=ltgsd_v[:, b, 65:66])
            else:
                nc.vector.tensor_scalar(mt[:], s_rep[:, i0:i0 + w],
                                        negd[:, b:b + 1], None, op0=Alu.is_ge)
            mask_tiles[(pair, b)] = mt

        def exp_tables(half):
            """d/s-derived exp tables for blocks 16*half .. 16*half+15."""
            bsl = slice(16 * half, 16 * half + 16)
            d_src = ltgsd_v[:, bsl, 65]
            s_src = ltgsd_v[:, bsl, 64]
            nc.scalar.activation(vuse[:, bsl], d_src, Act.Exp)
            nc.scalar.activation(v2use[:, bsl], d_src, Act.Exp, scale=0.2)
            nc.scalar.activation(rho[:, bsl], s_src, Act.Exp, scale=0.8)
            # halve the B± columns (exact mask algebra for sign masks)
            nc.vector.tensor_tensor(vuse[:, bsl], vuse[:, bsl],
                                    hmul[:, bsl], op=Alu.mult)
            nc.vector.tensor_tensor(v2use[:, bsl], v2use[:, bsl],
                                    hmul[:, bsl], op=Alu.mult)

        def r_build(b):
            ltg_b = ltgsd[:, 66 * b:66 * b + F]
            r0 = RC * b
            nc.vector.tensor_scalar(r_all[:, r0:r0 + F], ltg_b,
                                    vuse[:, b:b + 1], None, op0=Alu.mult)
            nc.vector.tensor_scalar(r_all[:, r0 + 65:r0 + 65 + F], ltg_b,
                                    v2use[:, b:b + 1], None, op0=Alu.mult)

        # ---- setup: load graph, transpose, project, s_rep, tables ----
        NC_ = 8  # graph chunks of 512 rows
        for c in range(NC_):
            g_sb = gp.tile([P, 4 * F_IN], f32, tag="g")
            nc.sync.dma_start(
                g_sb.rearrange("p (k f) -> p k f", k=4),
                graph_d[c * 512:(c + 1) * 512, :].rearrange(
                    "(k p) f -> p k f", p=P))
            gT_ps = sps.tile([F_IN, 512], f32, tag="tp")
            for k in range(4):
                nc.tensor.transpose(gT_ps[:, P * k:P * (k + 1)],
                                    g_sb[:, F_IN * k:F_IN * (k + 1)],
                                    identity[:])
            nc.scalar.copy(gT[:, c * 512:(c + 1) * 512], gT_ps[:])
            for k in range(4):
                b = 4 * c + k
                prj_ps = sps.tile([P, F + 2], f32, tag="pj")
                nc.tensor.matmul(prj_ps[:], gT[:, b * P:(b + 1) * P], wssd[:])
                nc.vector.tensor_copy(ltgsd[:, 66 * b:66 * (b + 1)], prj_ps[:])
            # s_rep chunk c: (ones ⊗ w_s)^T @ gT chunk
            bc_ps = sps.tile([P, 512], f32, tag="pj", name="bc_ps")
            nc.tensor.matmul(bc_ps[:], W1[:], gT[:, c * 512:(c + 1) * 512])
            nc.scalar.copy(s_rep[:, c * 512:(c + 1) * 512], bc_ps[:])
            nc.vector.tensor_scalar(negd[:, 4 * c:4 * c + 4],
                                    ltgsd_v[:, 4 * c:4 * c + 4, 65], -1.0,
                                    None, op0=Alu.mult)
            # pipelined prefill of pair-0/1 masks (pair p needs s_rep
            # chunks 2p..2p+1; block b needs negd/ltgsd from chunk b//4)
            if c >= 1:
                for b in range(4 * (c - 1), 4 * c):
                    emit_mask(0, b)
            if c >= 3:
                for b in range(4 * (c - 3), 4 * (c - 2)):
                    emit_mask(1, b)
            if c == NC_ - 1:
                for b in range(4 * c, 4 * c + 4):
                    emit_mask(0, b)
                for b in range(4 * (c - 2), 4 * c + 4):
                    emit_mask(1, b)
            if c % 4 == 3:
                exp_tables(c // 4)
                for b in range(16 * (c // 4), 16 * (c // 4) + 16):
                    r_build(b)
        # v / v2 columns of R (strided single ops; B± already halved)
        nc.vector.tensor_copy(r_v[:, :, 64], vuse[:])
        nc.vector.tensor_copy(r_v[:, :, 129], v2use[:])
        # RS = sum of B± (halved) R blocks, bf16 pairwise tree
        sc_blocks = [b for b in range(NB) if is_sc[b]]
        acc_tiles = [r_v[:, b, :] for b in sc_blocks]
        scratch = []
        while len(acc_tiles) > 1:
            nxt = []
            for i in range(0, len(acc_tiles) - 1, 2):
                dst = RS[:] if len(acc_tiles) == 2 else None
                if dst is None:
                    t_ = gp.tile([P, RC], bf16, tag=f"rs{len(scratch)}")
                    scratch.append(t_)
                    dst = t_[:]
                nc.vector.tensor_tensor(dst, acc_tiles[i], acc_tiles[i + 1],
                                        op=Alu.add)
                nxt.append(dst)
            if len(acc_tiles) % 2:
                nxt.append(acc_tiles[-1])
            acc_tiles = nxt

        # T2 totals: ones^T @ [R2 | v2] over all b, plus RS correction
        t2_ps = sps.tile([1, F + 1], f32, tag="pj", name="t2ps")
        for b in range(NB):
            r0 = RC * b
            nc.tensor.matmul(t2_ps[:], ones_col_bf[:],
                             r_all[:, r0 + 65:r0 + 130],
                             start=(b == 0), stop=False)
        nc.tensor.matmul(t2_ps[:], ones_col_bf[:], RS[:, 65:130],
                         start=False, stop=True)
        t2acc = gp.tile([1, F + 1], f32, tag="t2acc")
        nc.vector.tensor_copy(t2acc[:], t2_ps[:])
        t2rep_ps = sps.tile([P, F + 1], f32, tag="tp", name="t2rep_ps")
        nc.tensor.matmul(t2rep_ps[:], ones_row[:], t2acc[:])
        nc.vector.tensor_copy(t2rep[:], t2rep_ps[:])
        t2rep4 = persist.tile([P, ISUP * F], f32)
        for t in range(ISUP):
            nc.vector.tensor_copy(t2rep4[:, F * t:F * (t + 1)], t2rep[:, 0:F])

        # ---- main masked-matmul loop ----
        def epilogue(sup, acc):
            q4 = ep.tile([P, 4 * 65], f32, tag="q4", name="q4")
            q4_v = q4.rearrange("p (t c) -> p t c", c=65)
            qa4 = ep.tile([P, 4 * 65], f32, tag="qa4", name="qa4")
            for t in range(ISUP):
                i = sup * ISUP + t
                nc.vector.tensor_scalar(
                    qa4[:, 65 * t:65 * t + 65],
                    acc[:, 256 * t:256 * t + 65],
                    rho[:, i:i + 1], None, op0=Alu.mult)
            for t in range(ISUP):
                nc.vector.tensor_tensor(
                    q4[:, 65 * t:65 * t + 65],
                    qa4[:, 65 * t:65 * t + 65],
                    acc[:, 256 * t + 65:256 * t + 130], op=Alu.subtract)
            ob = ep.tile([P, ISUP * F], f32, tag="ob", name="ob")
            num4 = ep.tile([P, ISUP * F], f32, tag="num4", name="num4")
            num4_v = num4.rearrange("p (t c) -> p t c", c=F)
            t2rep4_v = t2rep4.rearrange("p (t c) -> p t c", c=F)
            nc.vector.tensor_tensor(num4_v[:, :, :], q4_v[:, :, 0:F],
                                    t2rep4_v[:, :, :], op=Alu.add)
            den4 = ep.tile([P, ISUP], f32, tag="den4", name="den4")
            nc.vector.tensor_tensor(
                den4[:], q4_v[:, :, 64],
                t2rep[:, F:F + 1].to_broadcast([P, ISUP]), op=Alu.add)
            rden4 = ep.tile([P, ISUP], f32, tag="rden4", name="rden4")
            nc.vector.reciprocal(rden4[:], den4[:])
            for t in range(ISUP):
                nc.vector.tensor_scalar(ob[:, F * t:F * (t + 1)],
                                        num4[:, F * t:F * (t + 1)],
                                        rden4[:, t:t + 1], None, op0=Alu.mult)
            nc.sync.dma_start(
                out_d[sup * 512:(sup + 1) * 512, :].rearrange(
                    "(t p) f -> p t f", p=P),
                ob.rearrange("p (t f) -> p t f", f=F))

        for pair in range(NPAIR):
            if pair + 1 < NPAIR:
                for b in range(NB):
                    if (pair + 1, b) not in mask_tiles:
                        emit_mask(pair + 1, b)
            mtiles = [mask_tiles.pop((pair, b)) for b in range(NB)]
            for half in range(2):
                sup = 2 * pair + half
                acc = accp.tile([P, 1024], f32, tag="acc", name="acc")
                for t in range(ISUP):
                    col = half * 512 + t * P
                    for b in range(NB):
                        r0 = RC * b
                        nc.tensor.matmul(
                            acc[:, 256 * t:256 * t + RC],
                            mtiles[b][:, col:col + P],
                            r_all[:, r0:r0 + RC],
                            start=(b == 0), stop=False)
                    nc.tensor.matmul(acc[:, 256 * t:256 * t + RC],
                                     ones_sq_bf[:], RS[:],
                                     start=False, stop=True)
                epilogue(sup, acc)

    nc.compile()
    return nc


def _get_nc():
    if "nc" not in _CACHE:
        _CACHE["nc"] = _build()
    return _CACHE["nc"]


def kernel(graph, W, a):
    from concourse.bass_utils import run_bass_kernel_spmd

    graph = np.ascontiguousarray(np.asarray(graph, dtype=np.float32))
    W = np.asarray(W, dtype=np.float32)
    a = np.asarray(a, dtype=np.float32)

    nc = _get_nc()
    in_maps = [
        {
            "graph": graph,
            "w": np.ascontiguousarray(W[h]),
            "a": np.ascontiguousarray(a[h].reshape(2, F)),
        }
        for h in range(H)
    ]
    trace = bool(int(os.environ.get("GAT_TRACE", "0")))
    res = run_bass_kernel_spmd(nc, in_maps, core_ids=list(range(H)), trace=trace)
    _CACHE["last_result"] = res
    return np.stack([res.results[h]["out"] for h in range(H)], axis=0)


# revision 22
# speedup vs baseline: 1.0355x; 1.0355x over previous
"""Multi-head GAT layer for Trainium2 — 8 heads sharded across 8 NeuronCores.

Per head h (N=4096 nodes, F=64 features):
    ltg   = graph @ W[h]                          [N, F]
    s     = ltg @ a_src,  d = ltg @ a_dst         [N]
    E     = leaky_relu(s[:, None] + d[None, :], 0.2)
    Alpha = softmax(E, axis=-1)
    out   = Alpha @ ltg

Decomposition: with z = s_i + d_j and M_ij = [z >= 0],

    exp(leaky_relu(z)) = M_ij e^{s_i} e^{d_j} + (1-M_ij) e^{.2 s_i} e^{.2 d_j}

so softmax reduces to one mask pass plus masked matmuls on the PE:

    eps1 = M @ [v.*ltg | v]      (v  = e^{d})
    eps2 = M @ [v2.*ltg | v2]    (v2 = e^{.2 d})
    q    = rho * eps1 - eps2     (rho = e^{.8 s}, per row i)
    out  = (q[:, :F] + T2) / (q[:, F] + t2)

[T2 | t2] are full column sums of [v2.*ltg | v2]; the complement path is
total-minus-masked with identical bf16 summands, so it is exact.

Masks are made on two engines, two supers (1024 i-columns) at a time:
  - DVE tensor_scalar(is_ge) -> {0,1} masks, blocks B0.
  - ScalarE activation(Sign, bias=d_j) -> {-1,0,+1} masks tau, blocks B±.
    Sign is present in every activation table set, so no table reloads.
    For B± blocks R is stored halved; M @ R = tau @ (R/2) + ones @ (R/2),
    handled by one extra ones-stationary matmul per accumulation group
    against RS = sum of the halved R blocks.  At z == 0 sign gives 0,
    i.e. the average of both paths — exact, since they coincide there.

Heads are fully independent: core h computes head h; no collectives.
"""

import os
from contextlib import ExitStack

import numpy as np

N, F_IN, F, H = 4096, 64, 64, 8
P = 128
NB = N // P           # 32 node blocks (j)
ISUP = 4              # i-blocks per super (one PSUM acc generation)
NSUP = NB // ISUP     # 8 supers
NPAIR = NSUP // 2     # mask tiles span a pair of supers
RC = 130              # R columns per j-block: [v.*ltg | v | v2.*ltg | v2]
# blocks with (b % 8) in SC_PAT get ScalarE sign masks; rest DVE is_ge
SC_PAT = tuple(int(c) for c in os.environ.get("GAT_SC_PAT", "036"))
_CACHE = {}


def _build():
    import concourse.bass as bass  # noqa: F401
    import concourse.mybir as mybir
    import concourse.tile as tile
    from concourse import bacc

    dt = mybir.dt
    f32 = dt.float32
    bf16 = dt.bfloat16
    Alu = mybir.AluOpType
    Act = mybir.ActivationFunctionType

    nc = bacc.Bacc("TRN2", debug=False, num_devices=H)
    graph_d = nc.dram_tensor("graph", [N, F_IN], f32, kind="ExternalInput").ap()
    w_d = nc.dram_tensor("w", [F_IN, F], f32, kind="ExternalInput").ap()
    a_d = nc.dram_tensor("a", [2, F], f32, kind="ExternalInput").ap()
    out_d = nc.dram_tensor("out", [N, F], f32, kind="ExternalOutput").ap()

    ident_d = nc.inline_tensor(np.eye(P, dtype=np.float32), name="ident")
    is_sc = [1 if (b % 8) in SC_PAT else 0 for b in range(NB)]
    hrow_d = nc.inline_tensor(
        np.repeat(np.where(np.array(is_sc), 0.5, 1.0)[None, :],
                  P, axis=0).astype(np.float32), name="hrow")

    with tile.TileContext(nc) as tc, ExitStack() as ctx:
        persist = ctx.enter_context(tc.tile_pool(name="persist", bufs=1))
        sps = ctx.enter_context(tc.tile_pool(name="sps", bufs=2, space="PSUM"))
        accp = ctx.enter_context(tc.tile_pool(name="acc", bufs=2, space="PSUM"))
        gp = ctx.enter_context(tc.tile_pool(name="gp", bufs=3))
        mp = ctx.enter_context(tc.tile_pool(name="mask", bufs=2))
        ep = ctx.enter_context(tc.tile_pool(name="ep", bufs=2))

        identity = persist.tile([P, P], f32)
        nc.sync.dma_start(identity[:], ident_d.ap())
        hmul = persist.tile([P, NB], f32)            # 0.5 on B±, 1.0 on B0
        nc.sync.dma_start(hmul[:], hrow_d.ap())
        ones_col_bf = persist.tile([P, 1], bf16)
        nc.vector.memset(ones_col_bf[:], 1.0)
        ones_row = persist.tile([1, P], f32)
        nc.vector.memset(ones_row[:], 1.0)
        ones64 = persist.tile([F_IN, P], f32)
        nc.vector.memset(ones64[:], 1.0)
        ones_sq_bf = persist.tile([P, P], bf16)      # stationary for RS matmul
        nc.vector.memset(ones_sq_bf[:], 1.0)

        # fused [W | w_s | w_d] rhs for the per-block projection matmul
        wssd = persist.tile([F_IN, F + 2], f32)
        nc.sync.dma_start(wssd[:, 0:F], w_d[:])
        a2_sb = persist.tile([F, 2], f32)
        nc.sync.dma_start(a2_sb[:], a_d.rearrange("t k -> k t"))

        gT = persist.tile([F_IN, N], f32)            # graph^T
        ltgsd = persist.tile([P, 66 * NB], f32)      # per b: ltg (64) | s | d
        negd = persist.tile([P, NB], f32)            # -d columns
        vuse = persist.tile([P, NB], f32)            # e^{d}   (halved on B±)
        v2use = persist.tile([P, NB], f32)           # e^{.2d} (halved on B±)
        rho = persist.tile([P, NB], f32)             # e^{.8 s}
        s_rep = persist.tile([P, N], bf16)           # s broadcast down partitions
        r_all = persist.tile([P, RC * NB], bf16)     # [R1|v|R2|v2] per b
        RS = persist.tile([P, RC], bf16)             # sum of B± (halved) R
        t2rep = persist.tile([P, F + 1], f32)        # [T2 | t2] bcast down parts

        ltgsd_v = ltgsd.rearrange("p (b c) -> p b c", c=66)
        r_v = r_all.rearrange("p (b c) -> p b c", c=RC)

        # W^T, then [w_s | w_d] = W^T.T @ a2
        wT_ps = sps.tile([F, F_IN], f32, tag="tp")
        nc.tensor.transpose(wT_ps[:], wssd[:, 0:F], identity[0:F_IN, 0:F_IN])
        wT_sb = gp.tile([F, F_IN], f32, tag="wT")
        nc.vector.tensor_copy(wT_sb[:], wT_ps[:])
        wsd_ps = sps.tile([F_IN, 2], f32, tag="pj")
        nc.tensor.matmul(wsd_ps[:], wT_sb[:], a2_sb[:])
        nc.vector.tensor_copy(wssd[:, F:F + 2], wsd_ps[:])
        # W1[k, p] = w_s[k]  (pre-broadcast stationary for s_rep chunks)
        W1 = persist.tile([F_IN, P], f32)
        nc.vector.tensor_scalar(W1[:], ones64[:], wssd[:, F:F + 1], None,
                                op0=Alu.mult)

        mask_tiles = {}

        def emit_mask(pair, b):
            i0 = pair * 2 * ISUP * P
            w = 2 * ISUP * P
            mt = mp.tile([P, w], bf16, tag=f"m{b}", name=f"mask{b}")
            if is_sc[b]:
                nc.scalar.activation(mt[:], s_rep[:, i0:i0 + w], Act.Sign,
                                     bias=ltgsd_v[:, b, 65:66])
            else:
                nc.vector.tensor_scalar(mt[:], s_rep[:, i0:i0 + w],
                                        negd[:, b:b + 1], None, op0=Alu.is_ge)
            mask_tiles[(pair, b)] = mt

        def exp_tables(half):
            """d/s-derived exp tables for blocks 16*half .. 16*half+15."""
            bsl = slice(16 * half, 16 * half + 16)
            d_src = ltgsd_v[:, bsl, 65]
            s_src = ltgsd_v[:, bsl, 64]
            nc.scalar.activation(vuse[:, bsl], d_src, Act.Exp)
            nc.scalar.activation(v2use[:, bsl], d_src, Act.Exp, scale=0.2)
            nc.scalar.activation(rho[:, bsl], s_src, Act.Exp, scale=0.8)
            # halve the B± columns (exact mask algebra for sign masks)
            nc.vector.tensor_tensor(vuse[:, bsl], vuse[:, bsl],
                                    hmul[:, bsl], op=Alu.mult)
            nc.vector.tensor_tensor(v2use[:, bsl], v2use[:, bsl],
                                    hmul[:, bsl], op=Alu.mult)

        def r_build(b):
            ltg_b = ltgsd[:, 66 * b:66 * b + F]
            r0 = RC * b
            nc.vector.tensor_scalar(r_all[:, r0:r0 + F], ltg_b,
                                    vuse[:, b:b + 1], None, op0=Alu.mult)
            nc.vector.tensor_scalar(r_all[:, r0 + 65:r0 + 65 + F], ltg_b,
                                    v2use[:, b:b + 1], None, op0=Alu.mult)

        # ---- setup: load graph, transpose, project, s_rep, tables ----
        NC_ = 8  # graph chunks of 512 rows
        for c in range(NC_):
            g_sb = gp.tile([P, 4 * F_IN], f32, tag="g")
            nc.sync.dma_start(
                g_sb.rearrange("p (k f) -> p k f", k=4),
                graph_d[c * 512:(c + 1) * 512, :].rearrange(
                    "(k p) f -> p k f", p=P))
            gT_ps = sps.tile([F_IN, 512], f32, tag="tp")
            for k in range(4):
                nc.tensor.transpose(gT_ps[:, P * k:P * (k + 1)],
                                    g_sb[:, F_IN * k:F_IN * (k + 1)],
                                    identity[:])
            nc.vector.tensor_copy(gT[:, c * 512:(c + 1) * 512], gT_ps[:])
            for k in range(4):
                b = 4 * c + k
                prj_ps = sps.tile([P, F + 2], f32, tag="pj")
                nc.tensor.matmul(prj_ps[:], gT[:, b * P:(b + 1) * P], wssd[:])
                nc.vector.tensor_copy(ltgsd[:, 66 * b:66 * (b + 1)], prj_ps[:])
            # s_rep chunk c: (ones ⊗ w_s)^T @ gT chunk
            bc_ps = sps.tile([P, 512], f32, tag="tp", name="bc_ps")
            nc.tensor.matmul(bc_ps[:], W1[:], gT[:, c * 512:(c + 1) * 512])
            nc.scalar.copy(s_rep[:, c * 512:(c + 1) * 512], bc_ps[:])
            nc.vector.tensor_scalar(negd[:, 4 * c:4 * c + 4],
                                    ltgsd_v[:, 4 * c:4 * c + 4, 65], -1.0,
                                    None, op0=Alu.mult)
            # pipelined prefill of pair-0/1 masks
            if c >= 1:
                for b in range(4 * (c - 1), 4 * c):
                    emit_mask(0, b)
            if c >= 3:
                for b in range(4 * (c - 3), 4 * (c - 2)):
                    emit_mask(1, b)
            if c == NC_ - 1:
                for b in range(4 * c, 4 * c + 4):
                    emit_mask(0, b)
                for b in range(4 * (c - 2), 4 * c + 4):
                    emit_mask(1, b)
            if c % 4 == 3:
                exp_tables(c // 4)
                for b in range(16 * (c // 4), 16 * (c // 4) + 16):
                    r_build(b)
        # v / v2 columns of R (strided single ops; B± already halved)
        nc.vector.tensor_copy(r_v[:, :, 64], vuse[:])
        nc.vector.tensor_copy(r_v[:, :, 129], v2use[:])
        # RS = sum of B± (halved) R blocks, bf16 pairwise tree
        sc_blocks = [b for b in range(NB) if is_sc[b]]
        acc_tiles = [r_v[:, b, :] for b in sc_blocks]
        scratch = []
        while len(acc_tiles) > 1:
            nxt = []
            for i in range(0, len(acc_tiles) - 1, 2):
                dst = RS[:] if len(acc_tiles) == 2 else None
                if dst is None:
                    t_ = gp.tile([P, RC], bf16, tag=f"rs{len(scratch)}")
                    scratch.append(t_)
                    dst = t_[:]
                nc.vector.tensor_tensor(dst, acc_tiles[i], acc_tiles[i + 1],
                                        op=Alu.add)
                nxt.append(dst)
            if len(acc_tiles) % 2:
                nxt.append(acc_tiles[-1])
            acc_tiles = nxt

        # T2 totals: ones^T @ [R2 | v2] over all b, plus RS correction
        t2_ps = sps.tile([1, F + 1], f32, tag="pj", name="t2ps")
        for b in range(NB):
            r0 = RC * b
            nc.tensor.matmul(t2_ps[:], ones_col_bf[:],
                             r_all[:, r0 + 65:r0 + 130],
                             start=(b == 0), stop=False)
        nc.tensor.matmul(t2_ps[:], ones_col_bf[:], RS[:, 65:130],
                         start=False, stop=True)
        t2acc = gp.tile([1, F + 1], f32, tag="t2acc")
        nc.vector.tensor_copy(t2acc[:], t2_ps[:])
        t2rep_ps = sps.tile([P, F + 1], f32, tag="tp", name="t2rep_ps")
        nc.tensor.matmul(t2rep_ps[:], ones_row[:], t2acc[:])
        nc.vector.tensor_copy(t2rep[:], t2rep_ps[:])
        t2rep4 = persist.tile([P, ISUP * F], f32)
        for t in range(ISUP):
            nc.vector.tensor_copy(t2rep4[:, F * t:F * (t + 1)], t2rep[:, 0:F])

        # ---- main masked-matmul loop ----
        def epilogue(sup, acc):
            q4 = ep.tile([P, 4 * 65], f32, tag="q4", name="q4")
            q4_v = q4.rearrange("p (t c) -> p t c", c=65)
            qa4 = ep.tile([P, 4 * 65], f32, tag="qa4", name="qa4")
            for t in range(ISUP):
                i = sup * ISUP + t
                nc.vector.tensor_scalar(
                    qa4[:, 65 * t:65 * t + 65],
                    acc[:, 256 * t:256 * t + 65],
                    rho[:, i:i + 1], None, op0=Alu.mult)
            for t in range(ISUP):
                nc.vector.tensor_tensor(
                    q4[:, 65 * t:65 * t + 65],
                    qa4[:, 65 * t:65 * t + 65],
                    acc[:, 256 * t + 65:256 * t + 130], op=Alu.subtract)
            ob = ep.tile([P, ISUP * F], f32, tag="ob", name="ob")
            num4 = ep.tile([P, ISUP * F], f32, tag="num4", name="num4")
            num4_v = num4.rearrange("p (t c) -> p t c", c=F)
            t2rep4_v = t2rep4.rearrange("p (t c) -> p t c", c=F)
            nc.vector.tensor_tensor(num4_v[:, :, :], q4_v[:, :, 0:F],
                                    t2rep4_v[:, :, :], op=Alu.add)
            den4 = ep.tile([P, ISUP], f32, tag="den4", name="den4")
            nc.vector.tensor_tensor(
                den4[:], q4_v[:, :, 64],
                t2rep[:, F:F + 1].to_broadcast([P, ISUP]), op=Alu.add)
            rden4 = ep.tile([P, ISUP], f32, tag="rden4", name="rden4")
            nc.vector.reciprocal(rden4[:], den4[:])
            for t in range(ISUP):
                nc.vector.tensor_scalar(ob[:, F * t:F * (t + 1)],
                                        num4[:, F * t:F * (t + 1)],
                                        rden4[:, t:t + 1], None, op0=Alu.mult)
            nc.sync.dma_start(
                out_d[sup * 512:(sup + 1) * 512, :].rearrange(
                    "(t p) f -> p t f", p=P),
                ob.rearrange("p (t f) -> p t f", f=F))

        for pair in range(NPAIR):
            if pair + 1 < NPAIR:
                for b in range(NB):
                    if (pair + 1, b) not in mask_tiles:
                        emit_mask(pair + 1, b)
            mtiles = [mask_tiles.pop((pair, b)) for b in range(NB)]
            for half in range(2):
                sup = 2 * pair + half
                acc = accp.tile([P, 1024], f32, tag="acc", name="acc")
                for t in range(ISUP):
                    col = half * 512 + t * P
                    for b in range(NB):
                        r0 = RC * b
                        nc.tensor.matmul(
                            acc[:, 256 * t:256 * t + RC],
                            mtiles[b][:, col:col + P],
                            r_all[:, r0:r0 + RC],
                            start=(b == 0), stop=False)
                    nc.tensor.matmul(acc[:, 256 * t:256 * t + RC],
                                     ones_sq_bf[:], RS[:],
                                     start=False, stop=True)
                epilogue(sup, acc)

    nc.compile()
    return nc


def _get_nc():
    if "nc" not in _CACHE:
        _CACHE["nc"] = _build()
    return _CACHE["nc"]


def kernel(graph, W, a):
    from concourse.bass_utils import run_bass_kernel_spmd

    graph = np.ascontiguousarray(np.asarray(graph, dtype=np.float32))
    W = np.asarray(W, dtype=np.float32)
    a = np.asarray(a, dtype=np.float32)

    nc = _get_nc()
    in_maps = [
        {
            "graph": graph,
            "w": np.ascontiguousarray(W[h]),
            "a": np.ascontiguousarray(a[h].reshape(2, F)),
        }
        for h in range(H)
    ]
    trace = bool(int(os.environ.get("GAT_TRACE", "0")))
    res = run_bass_kernel_spmd(nc, in_maps, core_ids=list(range(H)), trace=trace)
    _CACHE["last_result"] = res
    return np.stack([res.results[h]["out"] for h in range(H)], axis=0)


# revision 23
# speedup vs baseline: 1.0779x; 1.0410x over previous
"""Multi-head GAT layer for Trainium2 — 8 heads sharded across 8 NeuronCores.

Per head h (N=4096 nodes, F=64 features):
    ltg   = graph @ W[h]                          [N, F]
    s     = ltg @ a_src,  d = ltg @ a_dst         [N]
    E     = leaky_relu(s[:, None] + d[None, :], 0.2)
    Alpha = softmax(E, axis=-1)
    out   = Alpha @ ltg

Decomposition: with z = s_i + d_j and M_ij = [z >= 0],

    exp(leaky_relu(z)) = M_ij e^{s_i} e^{d_j} + (1-M_ij) e^{.2 s_i} e^{.2 d_j}

so softmax reduces to one mask pass plus masked matmuls on the PE:

    eps1 = M @ [v.*ltg | v]      (v  = e^{d})
    eps2 = M @ [v2.*ltg | v2]    (v2 = e^{.2 d})
    q    = rho * eps1 - eps2     (rho = e^{.8 s}, per row i)
    out  = (q[:, :F] + T2) / (q[:, F] + t2)

[T2 | t2] are full column sums of [v2.*ltg | v2]; the complement path is
total-minus-masked with identical bf16 summands, so it is exact.

Masks are made on two engines, two supers (1024 i-columns) at a time:
  - DVE tensor_scalar(is_ge) -> {0,1} masks, blocks B0.
  - ScalarE activation(Sign, bias=d_j) -> {-1,0,+1} masks tau, blocks B±.
    Sign is present in every activation table set, so no table reloads.
    For B± blocks R is stored halved; M @ R = tau @ (R/2) + ones @ (R/2),
    handled by one extra ones-stationary matmul per accumulation group
    against RS = sum of the halved R blocks.  At z == 0 sign gives 0,
    i.e. the average of both paths — exact, since they coincide there.

Heads are fully independent: core h computes head h; no collectives.
"""

import os
from contextlib import ExitStack

import numpy as np

N, F_IN, F, H = 4096, 64, 64, 8
P = 128
NB = N // P           # 32 node blocks (j)
ISUP = 4              # i-blocks per super (one PSUM acc generation)
NSUP = NB // ISUP     # 8 supers
NPAIR = NSUP // 2     # mask tiles span a pair of supers
RC = 130              # R columns per j-block: [v.*ltg | v | v2.*ltg | v2]
# blocks with (b % 8) in SC_PAT get ScalarE sign masks; rest DVE is_ge
SC_PAT = tuple(int(c) for c in os.environ.get("GAT_SC_PAT", "036"))
_CACHE = {}


def _build():
    import concourse.bass as bass  # noqa: F401
    import concourse.mybir as mybir
    import concourse.tile as tile
    from concourse import bacc

    dt = mybir.dt
    f32 = dt.float32
    bf16 = dt.bfloat16
    Alu = mybir.AluOpType
    Act = mybir.ActivationFunctionType

    nc = bacc.Bacc("TRN2", debug=False, num_devices=H)
    graph_d = nc.dram_tensor("graph", [N, F_IN], f32, kind="ExternalInput").ap()
    w_d = nc.dram_tensor("w", [F_IN, F], f32, kind="ExternalInput").ap()
    a_d = nc.dram_tensor("a", [2, F], f32, kind="ExternalInput").ap()
    out_d = nc.dram_tensor("out", [N, F], f32, kind="ExternalOutput").ap()

    ident_d = nc.inline_tensor(np.eye(P, dtype=np.float32), name="ident")
    is_sc = [1 if (b % 8) in SC_PAT else 0 for b in range(NB)]
    hrow_d = nc.inline_tensor(
        np.repeat(np.where(np.array(is_sc), 0.5, 1.0)[None, :],
                  P, axis=0).astype(np.float32), name="hrow")

    with tile.TileContext(nc) as tc, ExitStack() as ctx:
        persist = ctx.enter_context(tc.tile_pool(name="persist", bufs=1))
        sps = ctx.enter_context(tc.tile_pool(name="sps", bufs=2, space="PSUM"))
        accp = ctx.enter_context(tc.tile_pool(name="acc", bufs=2, space="PSUM"))
        gp = ctx.enter_context(tc.tile_pool(name="gp", bufs=3))
        mp = ctx.enter_context(tc.tile_pool(name="mask", bufs=2))
        ep = ctx.enter_context(tc.tile_pool(name="ep", bufs=2))

        identity = persist.tile([P, P], f32)
        nc.sync.dma_start(identity[:], ident_d.ap())
        hmul = persist.tile([P, NB], f32)            # 0.5 on B±, 1.0 on B0
        nc.sync.dma_start(hmul[:], hrow_d.ap())
        ones_col_bf = persist.tile([P, 1], bf16)
        nc.vector.memset(ones_col_bf[:], 1.0)
        ones_row = persist.tile([1, P], f32)
        nc.vector.memset(ones_row[:], 1.0)
        ones64 = persist.tile([F_IN, P], f32)
        nc.vector.memset(ones64[:], 1.0)
        ones_sq_bf = persist.tile([P, P], bf16)      # stationary for RS matmul
        nc.vector.memset(ones_sq_bf[:], 1.0)

        # fused [W | w_s | w_d] rhs for the per-block projection matmul
        wssd = persist.tile([F_IN, F + 2], f32)
        nc.sync.dma_start(wssd[:, 0:F], w_d[:])
        a2_sb = persist.tile([F, 2], f32)
        nc.sync.dma_start(a2_sb[:], a_d.rearrange("t k -> k t"))

        gT = persist.tile([F_IN, N], f32)            # graph^T
        ltgsd = persist.tile([P, 66 * NB], f32)      # per b: ltg (64) | s | d
        negd = persist.tile([P, NB], f32)            # -d columns
        vuse = persist.tile([P, NB], f32)            # e^{d}   (halved on B±)
        v2use = persist.tile([P, NB], f32)           # e^{.2d} (halved on B±)
        rho = persist.tile([P, NB], f32)             # e^{.8 s}
        s_rep = persist.tile([P, N], bf16)           # s broadcast down partitions
        r_all = persist.tile([P, RC * NB], bf16)     # [R1|v|R2|v2] per b
        RS = persist.tile([P, RC], bf16)             # sum of B± (halved) R
        t2rep = persist.tile([P, F + 1], f32)        # [T2 | t2] bcast down parts

        ltgsd_v = ltgsd.rearrange("p (b c) -> p b c", c=66)
        r_v = r_all.rearrange("p (b c) -> p b c", c=RC)

        # W^T, then [w_s | w_d] = W^T.T @ a2
        wT_ps = sps.tile([F, F_IN], f32, tag="tp")
        nc.tensor.transpose(wT_ps[:], wssd[:, 0:F], identity[0:F_IN, 0:F_IN])
        wT_sb = gp.tile([F, F_IN], f32, tag="wT")
        nc.vector.tensor_copy(wT_sb[:], wT_ps[:])
        wsd_ps = sps.tile([F_IN, 2], f32, tag="pj")
        nc.tensor.matmul(wsd_ps[:], wT_sb[:], a2_sb[:])
        nc.vector.tensor_copy(wssd[:, F:F + 2], wsd_ps[:])
        # W1[k, p] = w_s[k]  (pre-broadcast stationary for s_rep chunks)
        W1 = persist.tile([F_IN, P], f32)
        nc.vector.tensor_scalar(W1[:], ones64[:], wssd[:, F:F + 1], None,
                                op0=Alu.mult)

        mask_tiles = {}

        def emit_mask(pair, b):
            i0 = pair * 2 * ISUP * P
            w = 2 * ISUP * P
            mt = mp.tile([P, w], bf16, tag=f"m{b}", name=f"mask{b}")
            if is_sc[b]:
                nc.scalar.activation(mt[:], s_rep[:, i0:i0 + w], Act.Sign,
                                     bias=ltgsd_v[:, b, 65:66])
            else:
                nc.vector.tensor_scalar(mt[:], s_rep[:, i0:i0 + w],
                                        negd[:, b:b + 1], None, op0=Alu.is_ge)
            mask_tiles[(pair, b)] = mt

        def exp_tables(half):
            """d/s-derived exp tables for blocks 16*half .. 16*half+15."""
            bsl = slice(16 * half, 16 * half + 16)
            d_src = ltgsd_v[:, bsl, 65]
            s_src = ltgsd_v[:, bsl, 64]
            nc.scalar.activation(vuse[:, bsl], d_src, Act.Exp)
            nc.scalar.activation(v2use[:, bsl], d_src, Act.Exp, scale=0.2)
            nc.scalar.activation(rho[:, bsl], s_src, Act.Exp, scale=0.8)
            # halve the B± columns (exact mask algebra for sign masks)
            nc.vector.tensor_tensor(vuse[:, bsl], vuse[:, bsl],
                                    hmul[:, bsl], op=Alu.mult)
            nc.vector.tensor_tensor(v2use[:, bsl], v2use[:, bsl],
                                    hmul[:, bsl], op=Alu.mult)

        def r_build(b):
            ltg_b = ltgsd[:, 66 * b:66 * b + F]
            r0 = RC * b
            nc.vector.tensor_scalar(r_all[:, r0:r0 + F], ltg_b,
                                    vuse[:, b:b + 1], None, op0=Alu.mult)
            nc.vector.tensor_scalar(r_all[:, r0 + 65:r0 + 65 + F], ltg_b,
                                    v2use[:, b:b + 1], None, op0=Alu.mult)

        # ---- setup: load graph, transpose, project, s_rep, tables ----
        NC_ = 8  # graph chunks of 512 rows
        for c in range(NC_):
            g_sb = gp.tile([P, 4 * F_IN], f32, tag="g")
            nc.sync.dma_start(
                g_sb.rearrange("p (k f) -> p k f", k=4),
                graph_d[c * 512:(c + 1) * 512, :].rearrange(
                    "(k p) f -> p k f", p=P))
            gT_ps = sps.tile([F_IN, 512], f32, tag="tp")
            for k in range(4):
                nc.tensor.transpose(gT_ps[:, P * k:P * (k + 1)],
                                    g_sb[:, F_IN * k:F_IN * (k + 1)],
                                    identity[:])
            nc.scalar.copy(gT[:, c * 512:(c + 1) * 512], gT_ps[:])
            for k in range(4):
                b = 4 * c + k
                prj_ps = sps.tile([P, F + 2], f32, tag="pj")
                nc.tensor.matmul(prj_ps[:], gT[:, b * P:(b + 1) * P], wssd[:])
                nc.vector.tensor_copy(ltgsd[:, 66 * b:66 * (b + 1)], prj_ps[:])
            # s_rep chunk c: (ones ⊗ w_s)^T @ gT chunk
            bc_ps = sps.tile([P, 512], f32, tag="tp", name="bc_ps")
            nc.tensor.matmul(bc_ps[:], W1[:], gT[:, c * 512:(c + 1) * 512])
            nc.scalar.copy(s_rep[:, c * 512:(c + 1) * 512], bc_ps[:])
            nc.vector.tensor_scalar(negd[:, 4 * c:4 * c + 4],
                                    ltgsd_v[:, 4 * c:4 * c + 4, 65], -1.0,
                                    None, op0=Alu.mult)
            # pipelined prefill of pair-0 masks (needs s_rep chunks 0-1)
            if c >= 1:
                for b in range(4 * (c - 1), 4 * c):
                    emit_mask(0, b)
            if c == NC_ - 1:
                for b in range(4 * c, 4 * c + 4):
                    emit_mask(0, b)
            if c % 4 == 3:
                exp_tables(c // 4)
                for b in range(16 * (c // 4), 16 * (c // 4) + 16):
                    r_build(b)
        # v / v2 columns of R (strided single ops; B± already halved)
        nc.vector.tensor_copy(r_v[:, :, 64], vuse[:])
        nc.vector.tensor_copy(r_v[:, :, 129], v2use[:])
        # RS = sum of B± (halved) R blocks, bf16 pairwise tree
        sc_blocks = [b for b in range(NB) if is_sc[b]]
        acc_tiles = [r_v[:, b, :] for b in sc_blocks]
        scratch = []
        while len(acc_tiles) > 1:
            nxt = []
            for i in range(0, len(acc_tiles) - 1, 2):
                dst = RS[:] if len(acc_tiles) == 2 else None
                if dst is None:
                    t_ = gp.tile([P, RC], bf16, tag=f"rs{len(scratch)}")
                    scratch.append(t_)
                    dst = t_[:]
                nc.vector.tensor_tensor(dst, acc_tiles[i], acc_tiles[i + 1],
                                        op=Alu.add)
                nxt.append(dst)
            if len(acc_tiles) % 2:
                nxt.append(acc_tiles[-1])
            acc_tiles = nxt

        # T2 totals: ones^T @ [R2 | v2] over all b, plus RS correction
        t2_ps = sps.tile([1, F + 1], f32, tag="pj", name="t2ps")
        for b in range(NB):
            r0 = RC * b
            nc.tensor.matmul(t2_ps[:], ones_col_bf[:],
                             r_all[:, r0 + 65:r0 + 130],
                             start=(b == 0), stop=False)
        nc.tensor.matmul(t2_ps[:], ones_col_bf[:], RS[:, 65:130],
                         start=False, stop=True)
        t2acc = gp.tile([1, F + 1], f32, tag="t2acc")
        nc.vector.tensor_copy(t2acc[:], t2_ps[:])
        t2rep_ps = sps.tile([P, F + 1], f32, tag="tp", name="t2rep_ps")
        nc.tensor.matmul(t2rep_ps[:], ones_row[:], t2acc[:])
        nc.vector.tensor_copy(t2rep[:], t2rep_ps[:])
        t2rep4 = persist.tile([P, ISUP * F], f32)
        for t in range(ISUP):
            nc.vector.tensor_copy(t2rep4[:, F * t:F * (t + 1)], t2rep[:, 0:F])

        # ---- main masked-matmul loop ----
        def epilogue(sup, acc):
            q4 = ep.tile([P, 4 * 65], f32, tag="q4", name="q4")
            q4_v = q4.rearrange("p (t c) -> p t c", c=65)
            qa4 = ep.tile([P, 4 * 65], f32, tag="qa4", name="qa4")
            for t in range(ISUP):
                i = sup * ISUP + t
                nc.vector.tensor_scalar(
                    qa4[:, 65 * t:65 * t + 65],
                    acc[:, 256 * t:256 * t + 65],
                    rho[:, i:i + 1], None, op0=Alu.mult)
            for t in range(ISUP):
                nc.vector.tensor_tensor(
                    q4[:, 65 * t:65 * t + 65],
                    qa4[:, 65 * t:65 * t + 65],
                    acc[:, 256 * t + 65:256 * t + 130], op=Alu.subtract)
            ob = ep.tile([P, ISUP * F], f32, tag="ob", name="ob")
            num4 = ep.tile([P, ISUP * F], f32, tag="num4", name="num4")
            num4_v = num4.rearrange("p (t c) -> p t c", c=F)
            t2rep4_v = t2rep4.rearrange("p (t c) -> p t c", c=F)
            nc.vector.tensor_tensor(num4_v[:, :, :], q4_v[:, :, 0:F],
                                    t2rep4_v[:, :, :], op=Alu.add)
            den4 = ep.tile([P, ISUP], f32, tag="den4", name="den4")
            nc.vector.tensor_tensor(
                den4[:], q4_v[:, :, 64],
                t2rep[:, F:F + 1].to_broadcast([P, ISUP]), op=Alu.add)
            rden4 = ep.tile([P, ISUP], f32, tag="rden4", name="rden4")
            nc.vector.reciprocal(rden4[:], den4[:])
            for t in range(ISUP):
                nc.vector.tensor_scalar(ob[:, F * t:F * (t + 1)],
                                        num4[:, F * t:F * (t + 1)],
                                        rden4[:, t:t + 1], None, op0=Alu.mult)
            nc.sync.dma_start(
                out_d[sup * 512:(sup + 1) * 512, :].rearrange(
                    "(t p) f -> p t f", p=P),
                ob.rearrange("p (t f) -> p t f", f=F))

        for pair in range(NPAIR):
            if pair + 1 < NPAIR:
                for b in range(NB):
                    emit_mask(pair + 1, b)
            mtiles = [mask_tiles.pop((pair, b)) for b in range(NB)]
            for half in range(2):
                sup = 2 * pair + half
                acc = accp.tile([P, 1024], f32, tag="acc", name="acc")
                for t in range(ISUP):
                    col = half * 512 + t * P
                    for b in range(NB):
                        r0 = RC * b
                        nc.tensor.matmul(
                            acc[:, 256 * t:256 * t + RC],
                            mtiles[b][:, col:col + P],
                            r_all[:, r0:r0 + RC],
                            start=(b == 0), stop=False)
                    nc.tensor.matmul(acc[:, 256 * t:256 * t + RC],
                                     ones_sq_bf[:], RS[:],
                                     start=False, stop=True)
                epilogue(sup, acc)

    nc.compile()
    return nc


def _get_nc():
    if "nc" not in _CACHE:
        _CACHE["nc"] = _build()
    return _CACHE["nc"]


def kernel(graph, W, a):
    from concourse.bass_utils import run_bass_kernel_spmd

    graph = np.ascontiguousarray(np.asarray(graph, dtype=np.float32))
    W = np.asarray(W, dtype=np.float32)
    a = np.asarray(a, dtype=np.float32)

    nc = _get_nc()
    in_maps = [
        {
            "graph": graph,
            "w": np.ascontiguousarray(W[h]),
            "a": np.ascontiguousarray(a[h].reshape(2, F)),
        }
        for h in range(H)
    ]
    trace = bool(int(os.environ.get("GAT_TRACE", "0")))
    res = run_bass_kernel_spmd(nc, in_maps, core_ids=list(range(H)), trace=trace)
    _CACHE["last_result"] = res
    return np.stack([res.results[h]["out"] for h in range(H)], axis=0)


# revision 24
# speedup vs baseline: 1.2179x; 1.1298x over previous
"""Multi-head GAT layer for Trainium2 — 8 heads sharded across 8 NeuronCores.

Per head h (N=4096 nodes, F=64 features):
    ltg   = graph @ W[h]                          [N, F]
    s     = ltg @ a_src,  d = ltg @ a_dst         [N]
    E     = leaky_relu(s[:, None] + d[None, :], 0.2)
    Alpha = softmax(E, axis=-1)
    out   = Alpha @ ltg

Decomposition: with z = s_i + d_j and M_ij = [z >= 0],

    exp(leaky_relu(z)) = M_ij e^{s_i} e^{d_j} + (1-M_ij) e^{.2 s_i} e^{.2 d_j}

so softmax reduces to one mask pass plus masked matmuls on the PE:

    eps1 = M @ [v.*ltg | v]      (v  = e^{d})
    eps2 = M @ [v2.*ltg | v2]    (v2 = e^{.2 d})
    q    = rho * eps1 - eps2     (rho = e^{.8 s}, per row i)
    out  = (q[:, :F] + T2) / (q[:, F] + t2)

[T2 | t2] are full column sums of [v2.*ltg | v2]; the complement path is
total-minus-masked with identical bf16 summands, so it is exact.

Masks are made on two engines, two supers (1024 i-columns) at a time:
  - DVE tensor_scalar(is_ge) -> {0,1} masks, blocks B0.
  - ScalarE activation(Sign, bias=d_j) -> {-1,0,+1} masks tau, blocks B±.
    Sign is present in every activation table set, so no table reloads.
    For B± blocks R is stored halved; M @ R = tau @ (R/2) + ones @ (R/2),
    handled by one extra ones-stationary matmul per accumulation group
    against RS = sum of the halved R blocks.  At z == 0 sign gives 0,
    i.e. the average of both paths — exact, since they coincide there.

Heads are fully independent: core h computes head h; no collectives.
"""

import os
from contextlib import ExitStack

import numpy as np

N, F_IN, F, H = 4096, 64, 64, 8
P = 128
NB = N // P           # 32 node blocks (j)
ISUP = 4              # i-blocks per super (one PSUM acc generation)
NSUP = NB // ISUP     # 8 supers
NPAIR = NSUP // 2     # mask tiles span a pair of supers
RC = 130              # R columns per j-block: [v.*ltg | v | v2.*ltg | v2]
# blocks with (b % 8) in SC_PAT get ScalarE sign masks; rest DVE is_ge
SC_PAT = tuple(int(c) for c in os.environ.get("GAT_SC_PAT", "036"))
_CACHE = {}


def _build():
    import concourse.bass as bass  # noqa: F401
    import concourse.mybir as mybir
    import concourse.tile as tile
    from concourse import bacc

    dt = mybir.dt
    f32 = dt.float32
    bf16 = dt.bfloat16
    Alu = mybir.AluOpType
    Act = mybir.ActivationFunctionType

    nc = bacc.Bacc("TRN2", debug=False, num_devices=H)
    graph_d = nc.dram_tensor("graph", [N, F_IN], f32, kind="ExternalInput").ap()
    w_d = nc.dram_tensor("w", [F_IN, F], f32, kind="ExternalInput").ap()
    a_d = nc.dram_tensor("a", [2, F], f32, kind="ExternalInput").ap()
    out_d = nc.dram_tensor("out", [N, F], f32, kind="ExternalOutput").ap()

    ident_d = nc.inline_tensor(np.eye(P, dtype=np.float32), name="ident")
    is_sc = [1 if (b % 8) in SC_PAT else 0 for b in range(NB)]
    hrow_d = nc.inline_tensor(
        np.repeat(np.where(np.array(is_sc), 0.5, 1.0)[None, :],
                  P, axis=0).astype(np.float32), name="hrow")

    with tile.TileContext(nc) as tc, ExitStack() as ctx:
        persist = ctx.enter_context(tc.tile_pool(name="persist", bufs=1))
        sps = ctx.enter_context(tc.tile_pool(name="sps", bufs=2, space="PSUM"))
        accp = ctx.enter_context(tc.tile_pool(name="acc", bufs=2, space="PSUM"))
        gp = ctx.enter_context(tc.tile_pool(name="gp", bufs=3))
        mp = ctx.enter_context(tc.tile_pool(name="mask", bufs=2))
        ep = ctx.enter_context(tc.tile_pool(name="ep", bufs=2))

        identity = persist.tile([P, P], f32)
        nc.sync.dma_start(identity[:], ident_d.ap())
        identity_bf = persist.tile([P, P], bf16)
        nc.vector.tensor_copy(identity_bf[:], identity[:])
        hmul = persist.tile([P, NB], f32)            # 0.5 on B±, 1.0 on B0
        nc.sync.dma_start(hmul[:], hrow_d.ap())
        ones_col_bf = persist.tile([P, 1], bf16)
        nc.vector.memset(ones_col_bf[:], 1.0)
        ones_row = persist.tile([1, P], f32)
        nc.vector.memset(ones_row[:], 1.0)
        ones64 = persist.tile([F_IN, P], f32)
        nc.vector.memset(ones64[:], 1.0)
        ones_sq_bf = persist.tile([P, P], bf16)      # stationary for RS matmul
        nc.vector.memset(ones_sq_bf[:], 1.0)

        # fused [W | w_s | w_d] rhs for the per-block projection matmul
        wssd = persist.tile([F_IN, F + 2], f32)
        nc.sync.dma_start(wssd[:, 0:F], w_d[:])
        a2_sb = persist.tile([F, 2], f32)
        nc.sync.dma_start(a2_sb[:], a_d.rearrange("t k -> k t"))

        gT = persist.tile([F_IN, N], bf16)           # graph^T (bf16 hi)
        ltgsd = persist.tile([P, 66 * NB], f32)      # per b: ltg (64) | s | d
        negd = persist.tile([P, NB], f32)            # -d columns
        vuse = persist.tile([P, NB], f32)            # e^{d}   (halved on B±)
        v2use = persist.tile([P, NB], f32)           # e^{.2d} (halved on B±)
        rho = persist.tile([P, NB], f32)             # e^{.8 s}
        s_rep = persist.tile([P, N], bf16)           # s broadcast down partitions
        r_all = persist.tile([P, RC * NB], bf16)     # [R1|v|R2|v2] per b
        RS = persist.tile([P, RC], bf16)             # sum of B± (halved) R
        t2rep = persist.tile([P, F + 1], f32)        # [T2 | t2] bcast down parts

        ltgsd_v = ltgsd.rearrange("p (b c) -> p b c", c=66)
        r_v = r_all.rearrange("p (b c) -> p b c", c=RC)

        # W^T, then [w_s | w_d] = W^T.T @ a2
        wT_ps = sps.tile([F, F_IN], f32, tag="tp")
        nc.tensor.transpose(wT_ps[:], wssd[:, 0:F], identity[0:F_IN, 0:F_IN])
        wT_sb = gp.tile([F, F_IN], f32, tag="wT")
        nc.vector.tensor_copy(wT_sb[:], wT_ps[:])
        wsd_ps = sps.tile([F_IN, 2], f32, tag="pj")
        nc.tensor.matmul(wsd_ps[:], wT_sb[:], a2_sb[:])
        nc.vector.tensor_copy(wssd[:, F:F + 2], wsd_ps[:])
        # bf16 hi/lo split of [W | w_s | w_d] for fast bf16 projections
        wssd_bf = persist.tile([F_IN, F + 2], bf16)
        nc.vector.tensor_copy(wssd_bf[:], wssd[:])
        wssd_hi = persist.tile([F_IN, F + 2], f32)
        nc.vector.tensor_copy(wssd_hi[:], wssd_bf[:])
        wlo_bf = persist.tile([F_IN, F + 2], bf16)
        nc.vector.tensor_tensor(wlo_bf[:], wssd[:], wssd_hi[:],
                                op=Alu.subtract)
        # W1[k, p] = w_s[k]  (pre-broadcast stationary for s_rep chunks)
        W1 = persist.tile([F_IN, P], bf16)
        nc.vector.tensor_scalar(W1[:], ones64[:], wssd[:, F:F + 1], None,
                                op0=Alu.mult)

        mask_tiles = {}

        def emit_mask(pair, b):
            i0 = pair * 2 * ISUP * P
            w = 2 * ISUP * P
            mt = mp.tile([P, w], bf16, tag=f"m{b}", name=f"mask{b}")
            if is_sc[b] and pair > 0:
                nc.scalar.activation(mt[:], s_rep[:, i0:i0 + w], Act.Sign,
                                     bias=ltgsd_v[:, b, 65:66])
            elif is_sc[b]:
                # pair 0 stays on DVE: {0,2} mask vs halved R is exact
                nc.vector.tensor_scalar(mt[:], s_rep[:, i0:i0 + w],
                                        negd[:, b:b + 1], 2.0,
                                        op0=Alu.is_ge, op1=Alu.mult)
            else:
                nc.vector.tensor_scalar(mt[:], s_rep[:, i0:i0 + w],
                                        negd[:, b:b + 1], None, op0=Alu.is_ge)
            mask_tiles[(pair, b)] = mt

        def exp_tables(half):
            """d/s-derived exp tables for blocks 16*half .. 16*half+15."""
            bsl = slice(16 * half, 16 * half + 16)
            d_src = ltgsd_v[:, bsl, 65]
            s_src = ltgsd_v[:, bsl, 64]
            nc.scalar.activation(vuse[:, bsl], d_src, Act.Exp)
            nc.scalar.activation(v2use[:, bsl], d_src, Act.Exp, scale=0.2)
            nc.scalar.activation(rho[:, bsl], s_src, Act.Exp, scale=0.8)
            # halve the B± columns (exact mask algebra for sign masks)
            nc.vector.tensor_tensor(vuse[:, bsl], vuse[:, bsl],
                                    hmul[:, bsl], op=Alu.mult)
            nc.vector.tensor_tensor(v2use[:, bsl], v2use[:, bsl],
                                    hmul[:, bsl], op=Alu.mult)

        def r_build(b):
            ltg_b = ltgsd[:, 66 * b:66 * b + F]
            r0 = RC * b
            nc.vector.tensor_scalar(r_all[:, r0:r0 + F], ltg_b,
                                    vuse[:, b:b + 1], None, op0=Alu.mult)
            nc.vector.tensor_scalar(r_all[:, r0 + 65:r0 + 65 + F], ltg_b,
                                    v2use[:, b:b + 1], None, op0=Alu.mult)

        # ---- setup: load graph, transpose, project, s_rep, tables ----
        NC_ = 8  # graph chunks of 512 rows
        for c in range(NC_):
            g_sb = gp.tile([P, 4 * F_IN], f32, tag="g")
            nc.sync.dma_start(
                g_sb.rearrange("p (k f) -> p k f", k=4),
                graph_d[c * 512:(c + 1) * 512, :].rearrange(
                    "(k p) f -> p k f", p=P))
            ghi = gp.tile([P, 4 * F_IN], bf16, tag="ghi")
            nc.vector.tensor_copy(ghi[:], g_sb[:])
            gT_ps = sps.tile([F_IN, 512], bf16, tag="tp")
            for k in range(4):
                nc.tensor.transpose(gT_ps[:, P * k:P * (k + 1)],
                                    ghi[:, F_IN * k:F_IN * (k + 1)],
                                    identity_bf[:])
            nc.scalar.copy(gT[:, c * 512:(c + 1) * 512], gT_ps[:])
            for k in range(4):
                b = 4 * c + k
                prj_ps = sps.tile([P, F + 2], f32, tag="pj")
                nc.tensor.matmul(prj_ps[:], gT[:, b * P:(b + 1) * P],
                                 wssd_bf[:], start=True, stop=False)
                nc.tensor.matmul(prj_ps[:], gT[:, b * P:(b + 1) * P],
                                 wlo_bf[:], start=False, stop=True)
                nc.scalar.copy(ltgsd[:, 66 * b:66 * (b + 1)], prj_ps[:])
            # s_rep chunk c: (ones ⊗ w_s)^T @ gT chunk
            bc_ps = sps.tile([P, 512], f32, tag="tp", name="bc_ps")
            nc.tensor.matmul(bc_ps[:], W1[:], gT[:, c * 512:(c + 1) * 512])
            nc.scalar.copy(s_rep[:, c * 512:(c + 1) * 512], bc_ps[:])
            nc.vector.tensor_scalar(negd[:, 4 * c:4 * c + 4],
                                    ltgsd_v[:, 4 * c:4 * c + 4, 65], -1.0,
                                    None, op0=Alu.mult)
            # pipelined prefill of pair-0 masks (needs s_rep chunks 0-1)
            if c >= 1:
                for b in range(4 * (c - 1), 4 * c):
                    emit_mask(0, b)
            if c == NC_ - 1:
                for b in range(4 * c, 4 * c + 4):
                    emit_mask(0, b)
            if c % 4 == 3:
                exp_tables(c // 4)
                for b in range(16 * (c // 4), 16 * (c // 4) + 16):
                    r_build(b)
        # v / v2 columns of R (strided single ops; B± already halved)
        nc.vector.tensor_copy(r_v[:, :, 64], vuse[:])
        nc.vector.tensor_copy(r_v[:, :, 129], v2use[:])
        # RS = sum of B± (halved) R blocks, bf16 pairwise tree
        sc_blocks = [b for b in range(NB) if is_sc[b]]
        acc_tiles = [r_v[:, b, :] for b in sc_blocks]
        scratch = []
        while len(acc_tiles) > 1:
            nxt = []
            for i in range(0, len(acc_tiles) - 1, 2):
                dst = RS[:] if len(acc_tiles) == 2 else None
                if dst is None:
                    t_ = gp.tile([P, RC], bf16, tag=f"rs{len(scratch)}")
                    scratch.append(t_)
                    dst = t_[:]
                nc.vector.tensor_tensor(dst, acc_tiles[i], acc_tiles[i + 1],
                                        op=Alu.add)
                nxt.append(dst)
            if len(acc_tiles) % 2:
                nxt.append(acc_tiles[-1])
            acc_tiles = nxt

        # T2 totals: ones^T @ [R2 | v2] over all b, plus RS correction
        t2_ps = sps.tile([1, F + 1], f32, tag="pj", name="t2ps")
        for b in range(NB):
            r0 = RC * b
            nc.tensor.matmul(t2_ps[:], ones_col_bf[:],
                             r_all[:, r0 + 65:r0 + 130],
                             start=(b == 0), stop=False)
        nc.tensor.matmul(t2_ps[:], ones_col_bf[:], RS[:, 65:130],
                         start=False, stop=True)
        t2acc = gp.tile([1, F + 1], f32, tag="t2acc")
        nc.vector.tensor_copy(t2acc[:], t2_ps[:])
        t2rep_ps = sps.tile([P, F + 1], f32, tag="tp", name="t2rep_ps")
        nc.tensor.matmul(t2rep_ps[:], ones_row[:], t2acc[:])
        nc.vector.tensor_copy(t2rep[:], t2rep_ps[:])
        t2rep4 = persist.tile([P, ISUP * F], f32)
        for t in range(ISUP):
            nc.vector.tensor_copy(t2rep4[:, F * t:F * (t + 1)], t2rep[:, 0:F])

        # ---- main masked-matmul loop ----
        def epilogue(sup, acc):
            q4 = ep.tile([P, 4 * 65], f32, tag="q4", name="q4")
            q4_v = q4.rearrange("p (t c) -> p t c", c=65)
            qa4 = ep.tile([P, 4 * 65], f32, tag="qa4", name="qa4")
            for t in range(ISUP):
                i = sup * ISUP + t
                nc.vector.tensor_scalar(
                    qa4[:, 65 * t:65 * t + 65],
                    acc[:, 256 * t:256 * t + 65],
                    rho[:, i:i + 1], None, op0=Alu.mult)
            for t in range(ISUP):
                nc.vector.tensor_tensor(
                    q4[:, 65 * t:65 * t + 65],
                    qa4[:, 65 * t:65 * t + 65],
                    acc[:, 256 * t + 65:256 * t + 130], op=Alu.subtract)
            ob = ep.tile([P, ISUP * F], f32, tag="ob", name="ob")
            num4 = ep.tile([P, ISUP * F], f32, tag="num4", name="num4")
            num4_v = num4.rearrange("p (t c) -> p t c", c=F)
            t2rep4_v = t2rep4.rearrange("p (t c) -> p t c", c=F)
            nc.vector.tensor_tensor(num4_v[:, :, :], q4_v[:, :, 0:F],
                                    t2rep4_v[:, :, :], op=Alu.add)
            den4 = ep.tile([P, ISUP], f32, tag="den4", name="den4")
            nc.vector.tensor_tensor(
                den4[:], q4_v[:, :, 64],
                t2rep[:, F:F + 1].to_broadcast([P, ISUP]), op=Alu.add)
            rden4 = ep.tile([P, ISUP], f32, tag="rden4", name="rden4")
            nc.vector.reciprocal(rden4[:], den4[:])
            for t in range(ISUP):
                nc.vector.tensor_scalar(ob[:, F * t:F * (t + 1)],
                                        num4[:, F * t:F * (t + 1)],
                                        rden4[:, t:t + 1], None, op0=Alu.mult)
            nc.sync.dma_start(
                out_d[sup * 512:(sup + 1) * 512, :].rearrange(
                    "(t p) f -> p t f", p=P),
                ob.rearrange("p (t f) -> p t f", f=F))

        for pair in range(NPAIR):
            if pair + 1 < NPAIR:
                for b in range(NB):
                    emit_mask(pair + 1, b)
            mtiles = [mask_tiles.pop((pair, b)) for b in range(NB)]
            for half in range(2):
                sup = 2 * pair + half
                acc = accp.tile([P, 1024], f32, tag="acc", name="acc")
                for t in range(ISUP):
                    col = half * 512 + t * P
                    for b in range(NB):
                        r0 = RC * b
                        nc.tensor.matmul(
                            acc[:, 256 * t:256 * t + RC],
                            mtiles[b][:, col:col + P],
                            r_all[:, r0:r0 + RC],
                            start=(b == 0),
                            stop=(b == NB - 1 and pair == 0))
                    if pair > 0:
                        nc.tensor.matmul(acc[:, 256 * t:256 * t + RC],
                                         ones_sq_bf[:], RS[:],
                                         start=False, stop=True)
                epilogue(sup, acc)

    nc.compile()
    return nc


def _get_nc():
    if "nc" not in _CACHE:
        _CACHE["nc"] = _build()
    return _CACHE["nc"]


def kernel(graph, W, a):
    from concourse.bass_utils import run_bass_kernel_spmd

    graph = np.ascontiguousarray(np.asarray(graph, dtype=np.float32))
    W = np.asarray(W, dtype=np.float32)
    a = np.asarray(a, dtype=np.float32)

    nc = _get_nc()
    in_maps = [
        {
            "graph": graph,
            "w": np.ascontiguousarray(W[h]),
            "a": np.ascontiguousarray(a[h].reshape(2, F)),
        }
        for h in range(H)
    ]
    trace = bool(int(os.environ.get("GAT_TRACE", "0")))
    res = run_bass_kernel_spmd(nc, in_maps, core_ids=list(range(H)), trace=trace)
    _CACHE["last_result"] = res
    return np.stack([res.results[h]["out"] for h in range(H)], axis=0)
